# revision 1
# baseline (speedup 1.0000x reference)
"""Trainium2 Bass kernel for nn_Cross_Attention_Fourier.

Math: with ortho-normalized FFTs, fft2 -> q@k^H -> ifft2 collapses exactly:
  ifft2(fft2(q) @ conj(fft2(k))^T) = (q @ k^T) @ J,  J: j -> (-j) mod n
so the block is plain attention with scores |q@k^T|, softmax/sqrt(d), applied
to row-flipped v.  No complex arithmetic.

Sharding (8 cores): core c -> sample b = c//2, query-token half (c%2)*512.
Each core computes LN+QKV for its slice (keys/values for the whole sample),
8 heads of attention; the FiLM t-vector is sharded 8-way and AllReduced
early (hidden under attention); the sample-global mean/std needs a tiny
[1,2] PAIRWISE AllReduce (cores 2c,2c+1 share a sample); then output
projection + feed-forward on its 512 tokens.

Key perf structure (rewrite of the 357us baseline; ~200us measured):
 - All big-GEMM operands (weights, raw inputs, k/q/v/ex/attention outputs)
   are bfloat16: 4x faster LDWEIGHTS via fast-weight-load (fp32 has none,
   and every matmul reloads its stationary), halved DMA, and 2x DVE modes
   on bf16 elementwise ops.  PSUM accumulation and all LN/softmax row math
   stay fp32; folded LN rank-1 corrections stay float32r.
 - The flipped-v input reuses the kv LN stats (mean/inv rows are flips of
   the kv rows) - no second stats pass; softmax /sqrt(64) is folded into
   wv host-side (wv/8), and the 65th all-ones v column yields each head's
   softmax denominator for free.  Per head: one broadcast matmul + one
   reciprocal_approx_fast (PSUM->SBUF, ~5x faster than reciprocal()) +
   one multiply straight out of the po PSUM normalizes the head.
 - scores for a kt-pair land in one 2-bank PSUM tile so a single
   [128,1024] DVE int-AND computes |S| for both; exp runs merged on ACT;
   the last pair's abs runs on ACT to balance the two engines.
 - global-norm stats are per-pair DVE reduce columns (no PSUM banks held
   across attention), freeing banks for score/po pipelining (stps 2x2,
   po x3).
 - AllReduce #2 is pairwise ([[0,1],[2,3],...], 8-byte payload), and the
   whole wo/m1 side of the tail is hoisted BEFORE it:  with
   y = inv_sd*A + cb(x)1 for A=(wo.std_t)^T@outT, cb = wo^T mean_t + bo
   - mu*inv_sd*wo^T std_t, both A and B=m1^T@A (plus the tiny wo^T/m1^T
   FiLM-vector projections) are AllReduce-independent and overlap its
   latency; post-AR2 only scalar fixups, LN2 stats on reconstructed y,
   gelu and the m2 GEMM remain.
 - wk|wq|wv and wo|m1|m2 are host-packed into two [512,1536] tensors and
   the small FiLM/constant tensors issue first: DMA_DIRECT2D dispatch on
   the sync engine is ~700ns each and was serializing startup.
"""

import numpy as np

import concourse.bass as bass
import concourse.bacc as bacc
import concourse.mybir as mybir
import concourse.tile as tile
from concourse.bass_utils import run_bass_kernel_spmd

AF = mybir.ActivationFunctionType
ALU = mybir.AluOpType
F32 = mybir.dt.float32
F32R = mybir.dt.float32r
BF = mybir.dt.bfloat16
I32 = mybir.dt.int32

N_CORES = 8
B = 4
NT = 1024          # tokens (keys)
TQ = 512           # query tokens per core
D = 512            # model dim
H = 8              # heads
DH = 64            # head dim
DC = 4             # dim chunks of 128
KT = 8             # key-token tiles of 128
E2 = 1024          # 2*D (FiLM width)
NEL = float(NT * D)
ABS_ACT_KT = (3, 7)  # which kt of each head the Scalar engine handles


def f32(ap):
    return ap.bitcast(F32)


def r32(ap):
    return ap.bitcast(F32R)


def _build_nc(gelu_mode="hw", has_bias=False):
    global _GELU_FUNC
    _GELU_FUNC = AF.Gelu if gelu_mode == "hw" else AF.Tanh
    nc = bacc.Bacc("TRN2", target_bir_lowering=False, debug=False,
                   num_devices=N_CORES)

    def din(name, shape, dt=F32):
        return nc.dram_tensor(name, shape, dt, kind="ExternalInput").ap()

    t = dict(
        xq=din("xq", [D, TQ], BF),
        xkv=din("xkv", [D, NT], BF),
        xv=din("xv", [D, NT], BF),
        wkqv=din("wkqv", [D, 3 * D], BF),
        womm=din("womm", [D, 3 * D], BF),
        nws=din("nws", [4, D]),          # -colsum rows: q,k,v,m1
        w1e=din("w1e", [D, 128]),
        w2e=din("w2e", [128, E2]),
        b1e=din("b1e", [128, 1]),
        emb=din("emb", [D, B]),
        bo=din("bo", [128, DC]),
        b2=din("b2", [128, DC]),
        b2e=din("b2e", [1, E2]),
        sel4=din("sel4", [B, 1]),
        ones8=din("ones8", [128, H]),
        pbias=din("pbias", [4, D]),      # bq,bk,bv,b1 rows (bias mode)
    )
    t["out"] = nc.dram_tensor("out", [D, TQ], F32, kind="ExternalOutput").ap()
    t["has_bias"] = has_bias

    with tile.TileContext(nc) as tc:
        _emit(nc, tc, t)
    nc.compile()
    return nc


def _emit(nc, tc, t):
    xq, xkv, xv = t["xq"], t["xkv"], t["xv"]
    wkqv, womm = t["wkqv"], t["womm"]
    w1e, w2e, b1e, emb = t["w1e"], t["w2e"], t["b1e"], t["emb"]
    nws, bo, b2, b2e = t["nws"], t["bo"], t["b2"], t["b2e"]
    sel4, ones8, pbias, out = t["sel4"], t["ones8"], t["pbias"], t["out"]
    has_bias = t["has_bias"]

    from contextlib import ExitStack
    ctx = ExitStack()
    with ctx:
        cpool = ctx.enter_context(tc.tile_pool(name="const", bufs=1))
        rowpool = ctx.enter_context(tc.tile_pool(name="rows", bufs=1))
        outpool = ctx.enter_context(tc.tile_pool(name="outp", bufs=1))
        dpool = ctx.enter_context(tc.tile_pool(name="dram", bufs=1, space="DRAM"))

        attnpool = ctx.enter_context(tc.tile_pool(name="attn", bufs=1))

        raw_ctx = ExitStack()
        rawpool = raw_ctx.enter_context(tc.tile_pool(name="rawp", bufs=6))

        def load_cols(src, n, tag, pool, dt=F32R):
            tiles = []
            for j in range(n):
                tl = pool.tile([128, src.shape[1]], dt, tag=f"{tag}{j}",
                               name=f"{tag}{j}")
                nc.sync.dma_start(tl[:], src[j * 128:(j + 1) * 128, :].bitcast(dt))
                tiles.append(tl)
            return tiles

        raw_kv = []
        for j in range(DC):
            rw = rawpool.tile([128, NT], BF, tag="raw", bufs=8,
                              name=f"rwkv{j}")
            nc.sync.dma_start(rw[:], xkv[j * 128:(j + 1) * 128, :])
            raw_kv.append(rw)

        # ---- constants -------------------------------------------------
        ones_row = rowpool.tile([1, 128], F32R, tag="ones_row")
        nc.sync.dma_start(ones_row[:], ones8[:, 0:1].rearrange("p x -> x p").bitcast(F32R))
        ones_col = rowpool.tile([128, 1], F32R, tag="ones_col")
        nc.sync.dma_start(ones_col[:], ones8[:, 0:1].bitcast(F32R))
        onesf = rowpool.tile([1, 128], F32, tag="onesf")
        nc.sync.dma_start(onesf[:], ones8[:, 0:1].rearrange("p x -> x p"))
        ones_col_bf = rowpool.tile([128, 1], BF, tag="ones_col_bf")
        nc.vector.memset(ones_col_bf[:], 1.0)
        sel_sb = rowpool.tile([B, 1], F32R, tag="sel")
        nc.sync.dma_start(sel_sb[:], sel4[:].bitcast(F32R))

        nws_sb, pb_sb = [], []
        for r in range(4):
            nt_ = rowpool.tile([1, D], F32R, tag=f"nws{r}", name=f"nws{r}")
            nc.sync.dma_start(nt_[:], nws[r:r + 1, :].bitcast(F32R))
            nws_sb.append(nt_)
            if has_bias:
                pt_ = rowpool.tile([1, D], F32R, tag=f"pb{r}", name=f"pb{r}")
                nc.sync.dma_start(pt_[:], pbias[r:r + 1, :].bitcast(F32R))
                pb_sb.append(pt_)

        def srt(w, name):
            return rowpool.tile([1, w], F32, tag="scratchrow", bufs=3,
                                name=name)[:, 0:w]

        def scw(name):
            return rowpool.tile([128, 1], F32, tag="scw", bufs=8, name=name)[:]

        w1e_t = load_cols(w1e, DC, "w1e", cpool, dt=F32R)
        w2e_t = load_cols(w2e, 1, "w2e", cpool)
        emb_t = load_cols(emb, DC, "emb", cpool, dt=F32R)
        b1e_sb = rowpool.tile([128, 1], F32, tag="b1e")
        nc.sync.dma_start(b1e_sb[:], b1e[:])
        bias_sb = {}
        for nm, src in [("bo", bo), ("b2", b2)]:
            tl = rowpool.tile([128, DC], F32, tag=nm, name=nm)
            nc.sync.dma_start(tl[:], src[:])
            bias_sb[nm] = tl

        ar1_in_d = dpool.tile([B, E2], F32, tag="ar1_in_d")
        ar1_out_d = dpool.tile([B, E2], F32, tag="ar1_out_d")
        ar2_in_d = dpool.tile([1, 2], F32, tag="ar2_in_d")
        ar2_out_d = dpool.tile([1, 2], F32, tag="ar2_out_d")

        # ---- FiLM partial + early collective #1 ------------------------
        with tc.tile_pool(name="psF", bufs=1, space="PSUM") as psF:
            ps_f = psF.tile([128, B], F32, tag="ftp_f")
            for j in range(DC):
                nc.tensor.matmul(ps_f[:], w1e_t[j][:], emb_t[j][:],
                                 start=(j == 0), stop=(j == DC - 1))
            silu_sb = rowpool.tile([128, B], F32R, tag="silu")
            if _GELU_FUNC == AF.Gelu:   # hw mode: fused Silu
                nc.scalar.activation(silu_sb[:], ps_f[:], AF.Silu,
                                     bias=b1e_sb[:])
            else:                        # sim lacks Silu: sigmoid * x
                xb_sb = rowpool.tile([128, B], F32, tag="xb")
                nc.scalar.activation(xb_sb[:], ps_f[:], AF.Identity,
                                     bias=b1e_sb[:])
                sg_sb = rowpool.tile([128, B], F32, tag="sg")
                nc.scalar.activation(sg_sb[:], xb_sb[:], AF.Sigmoid)
                nc.vector.tensor_tensor(silu_sb[:], xb_sb[:], sg_sb[:],
                                        op=ALU.mult)
            ps_t = psF.tile([B, E2], F32, tag="ftp_t")
            for a in range(2):
                nc.tensor.matmul(ps_t[:, a * 512:(a + 1) * 512], silu_sb[:],
                                 w2e_t[0][:, a * 512:(a + 1) * 512],
                                 start=True, stop=True)
            ar1_in = rowpool.tile([B, E2], F32, tag="ar1in")
            nc.vector.tensor_copy(ar1_in[:], ps_t[:])
            nc.sync.dma_start(ar1_in_d[:], ar1_in[:])
        nc.gpsimd.collective_compute(
            "AllReduce", ALU.add, replica_groups=[list(range(N_CORES))],
            ins=[ar1_in_d.opt()], outs=[ar1_out_d.opt()])
        ar1_sb = rowpool.tile([B, E2], F32R, tag="ar1sb")
        nc.sync.dma_start(ar1_sb[:], ar1_out_d[:].bitcast(F32R))

        raw_q = []
        for j in range(DC):
            rw = rawpool.tile([128, TQ], BF, tag="rawq", bufs=4, name=f"rwq{j}")
            nc.sync.dma_start(rw[:], xq[j * 128:(j + 1) * 128, :])
            raw_q.append(rw)
        raw_v = []
        for j in range(DC):
            rw = rawpool.tile([128, NT], BF, tag="raw", bufs=8,
                              name=f"rwv{j}")
            nc.sync.dma_start(rw[:], xv[j * 128:(j + 1) * 128, :])
            raw_v.append(rw)


        qTp = [attnpool.tile([128, TQ], BF, tag=f"qTp{p}", name=f"qTp{p}")
               for p in range(DC)]
        kTp = [attnpool.tile([128, NT], BF, tag=f"kTp{p}", name=f"kTp{p}")
               for p in range(DC)]
        vt = [attnpool.tile([128, H * 65], BF, tag=f"vt{t_}", name=f"vt{t_}")
              for t_ in range(KT)]
        ivc_v = attnpool.tile([128, KT], F32, tag="ivc_v")

        # ---- LN stats + folded projections -----------------------------
        with tc.tile_pool(name="lnsq", bufs=3) as lnsq, \
             tc.tile_pool(name="lnrows", bufs=1) as lnrows, \
             tc.tile_pool(name="psLN", bufs=1, space="PSUM") as psLN:

            def ln_stats(raws, T, mtag):
                """returns (mrow[f32r], inv_row[f32], sd_row[f32r]|None)."""
                sqs = []
                ps_s = psLN.tile([1, T], F32, tag="lnS", bufs=1, name="ps_s")
                ps_q = psLN.tile([1, T], F32, tag="lnQ", bufs=1, name="ps_q")
                for j in range(DC):
                    sq = lnsq.tile([128, T], BF, tag="sq", name="sq")
                    nc.vector.tensor_tensor(sq[:], raws[j][:],
                                            raws[j][:], op=ALU.mult)
                    sqs.append(sq)
                for a in range(T // 512):
                    sl = slice(a * 512, (a + 1) * 512)
                    for j in range(DC):
                        nc.tensor.matmul(ps_s[:, sl], ones_col_bf[:],
                                         raws[j][:, sl], start=(j == 0),
                                         stop=(j == DC - 1),
                                         skip_group_check=True)
                        nc.tensor.matmul(ps_q[:, sl], ones_col_bf[:],
                                         sqs[j][:, sl], start=(j == 0),
                                         stop=(j == DC - 1),
                                         skip_group_check=True)
                mrow = lnrows.tile([1, T], F32R, tag=f"mrow_{mtag}",
                                   name=f"mrow_{mtag}")
                nc.vector.tensor_scalar_mul(mrow[:], ps_s[:], 1.0 / D)
                var = srt(T, "var")
                nc.vector.tensor_scalar(var, ps_q[:], 1.0 / D, 1e-5,
                                        op0=ALU.mult, op1=ALU.add)
                msq = srt(T, "msq")
                nc.scalar.activation(msq, f32(mrow[:]), AF.Square)
                nc.vector.tensor_tensor(var, var, msq, op=ALU.subtract)
                if has_bias:
                    sd = lnrows.tile([1, T], F32R, tag=f"sd_{mtag}",
                                     name=f"sd_{mtag}")[:]
                else:
                    sd = lnrows.tile([1, T], F32R, tag="sdrot", bufs=2,
                                     name=f"sd_{mtag}")[:, 0:T]
                nc.scalar.activation(sd, var, AF.Sqrt)
                inv = lnrows.tile([1, T], F32, tag=f"inv_{mtag}",
                                  name=f"inv_{mtag}")[:]
                nc.vector.reciprocal_approx_fast(inv, f32(sd))
                return mrow, inv, sd

            def ivc_fill(inv_row, dst):
                """transpose a [1,128*KTn] f32 row into [128, KTn] columns."""
                ktn = dst.shape[1]
                pp = psLN.tile([128, KT], F32, tag="pivc", bufs=1, name="pivc")
                for ti in range(ktn):
                    nc.tensor.matmul(pp[:, ti:ti + 1],
                                     inv_row[0:1, ti * 128:(ti + 1) * 128],
                                     onesf[0:1, 0:1], is_transpose=True,
                                     skip_group_check=True)
                nc.vector.tensor_copy(dst[:], pp[:, 0:ktn])

            with tc.tile_pool(name="zw1", bufs=1) as zw1:
                wkqv_t = load_cols(wkqv, DC, "wkqv", zw1, dt=BF)
                wk_t = [w[:, 0:D] for w in wkqv_t]
                wq_t = [w[:, D:2 * D] for w in wkqv_t]
                wv_t = [w[:, 2 * D:3 * D] for w in wkqv_t]

                # ---- kv stats; v stats are flips of kv stats ----
                mrow_kv, inv_kv, sd_kv = ln_stats(raw_kv, NT, "kv")
                ivb_kv = lnrows.tile([128, NT], F32, tag="ivb_kv")
                for a in range(2):
                    asl = slice(a * 512, (a + 1) * 512)
                    ppk = psLN.tile([128, 512], F32, tag="pk1", bufs=3,
                                    name="ppk")
                    nc.tensor.matmul(ppk[:], onesf[0:1, :], inv_kv[0:1, asl],
                                     start=True, stop=True)
                    nc.scalar.activation(ivb_kv[:, asl], ppk[:], AF.Identity)
                mrow_v = lnrows.tile([1, NT], F32R, tag="mrow_v")
                nc.scalar.activation(mrow_v[:, 0:1],
                                     f32(mrow_kv[:, 0:1]), AF.Identity)
                nc.scalar.activation(mrow_v[0:1, 1:NT],
                                     f32(mrow_kv[0:1, NT - 1:0:-1]),
                                     AF.Identity)
                inv_v = lnrows.tile([1, NT], F32, tag="inv_v")
                nc.scalar.activation(inv_v[:, 0:1], inv_kv[:, 0:1],
                                     AF.Identity)
                nc.scalar.activation(inv_v[0:1, 1:NT],
                                     inv_kv[0:1, NT - 1:0:-1], AF.Identity)
                ivc_fill(inv_v, ivc_v)
                if has_bias:
                    sd_v = lnrows.tile([1, NT], F32R, tag="sd_v")
                    nc.vector.tensor_copy(sd_v[:, 0:1], sd_kv[:, 0:1])
                    nc.vector.tensor_copy(f32(sd_v[0:1, 1:NT]),
                                          f32(sd_kv[0:1, NT - 1:0:-1]))

                # ---- k (LN inv-std applied at evacuation) ----
                for mi in range(DC):
                    msl = slice(mi * 128, (mi + 1) * 128)
                    for a in range(NT // 512):
                        sl = slice(a * 512, (a + 1) * 512)
                        pp = psLN.tile([128, 512], F32, tag="pk1", bufs=3,
                                       name="pp")
                        for j in range(DC):
                            nc.tensor.matmul(pp[:], wk_t[j][:, msl],
                                             raw_kv[j][:, sl],
                                             start=(j == 0), stop=False)
                        nc.tensor.matmul(pp[:], nws_sb[1][:, msl],
                                         mrow_kv[:, sl], start=False,
                                         stop=not has_bias)
                        if has_bias:
                            nc.tensor.matmul(pp[:], pb_sb[1][:, msl],
                                             sd_kv[:, sl], start=False,
                                             stop=True)
                        nc.vector.tensor_tensor(kTp[mi][:, sl], pp[:],
                                                ivb_kv[:, sl], op=ALU.mult)

                # ---- q ----
                mrow_q, inv_q, sd_q = ln_stats(raw_q, TQ, "q")
                ivb_q = lnrows.tile([128, TQ], F32, tag="ivb_q")
                ppb = psLN.tile([128, 512], F32, tag="pk1", bufs=3, name="ppb")
                nc.tensor.matmul(ppb[:], onesf[0:1, :], inv_q[:],
                                 start=True, stop=True)
                nc.scalar.activation(ivb_q[:], ppb[:], AF.Identity)
                for mi in range(DC):
                    msl = slice(mi * 128, (mi + 1) * 128)
                    pp = psLN.tile([128, 512], F32, tag="pk1", bufs=3,
                                   name="pp")
                    for j in range(DC):
                        nc.tensor.matmul(pp[:], wq_t[j][:, msl], raw_q[j][:],
                                         start=(j == 0), stop=False)
                    nc.tensor.matmul(pp[:], nws_sb[0][:, msl], mrow_q[:],
                                     start=False, stop=not has_bias)
                    if has_bias:
                        nc.tensor.matmul(pp[:], pb_sb[0][:, msl], sd_q[:],
                                         start=False, stop=True)
                    nc.vector.tensor_tensor(qTp[mi][:], pp[:],
                                            ivb_q[:], op=ALU.mult)

                # ---- v (from host-flipped raw; wv carries the /8 fold) ----
                for ti in range(KT):
                    tsl = slice(ti * 128, (ti + 1) * 128)
                    pv = psLN.tile([128, D], F32, tag="pk1", bufs=3, name="pv")
                    for j in range(DC):
                        nc.tensor.matmul(pv[:], raw_v[j][:, tsl], wv_t[j][:],
                                         start=(j == 0), stop=False)
                    nc.tensor.matmul(pv[:], mrow_v[:, tsl], nws_sb[2][:],
                                     start=False, stop=not has_bias)
                    if has_bias:
                        nc.tensor.matmul(pv[:], sd_v[:, tsl], pb_sb[2][:],
                                         start=False, stop=True)
                    vw = vt[ti][:].rearrange("p (h x) -> p h x", h=H)
                    nc.scalar.activation(
                        vw[:, :, 0:DH],
                        pv[:].rearrange("p (h x) -> p h x", h=H),
                        AF.Identity, scale=ivc_v[:, ti:ti + 1])
                    nc.vector.memset(vw[:, :, DH:DH + 1], 1.0)

        raw_ctx.close()

        # weights for the tail; DMA overlaps attention
        wpool2 = ctx.enter_context(tc.tile_pool(name="w2", bufs=1))
        womm_t = load_cols(womm, DC, "womm", wpool2, dt=BF)
        wo_t = [w[:, 0:D] for w in womm_t]
        m1_t = [w[:, D:2 * D] for w in womm_t]
        m2_t = [w[:, 2 * D:3 * D] for w in womm_t]

        # ---- attention --------------------------------------------------
        outT = [outpool.tile([128, TQ], BF, tag=f"outT{j}", name=f"outT{j}")
                for j in range(DC)]
        tailrows = ctx.enter_context(tc.tile_pool(name="tailrows", bufs=1))
        b2e_sb = tailrows.tile([1, E2], F32, tag="b2e")
        nc.sync.dma_start(b2e_sb[:], b2e[:])
        mean_t = tailrows.tile([1, 512], F32, tag="mean_t")
        std_t = tailrows.tile([1, 512], F32, tag="std_t")
        stc = tailrows.tile([128, DC], F32, tag="stc")
        msrall = tailrows.tile([128, 2 * DC], BF, tag="msrall")
        dnr = [tailrows.tile([1, TQ], F32R, tag=f"dnr{h}", name=f"dnr{h}")
               for h in range(H)]

        def emit_t_processing(psA):
            ps_sel = [psA.tile([1, 512], F32, tag="stps", bufs=2,
                               name=f"ps_sel{a}") for a in range(2)]
            for a in range(2):
                nc.tensor.matmul(ps_sel[a][:], sel_sb[:],
                                 ar1_sb[:, a * 512:(a + 1) * 512],
                                 start=True, stop=True)
            nc.vector.tensor_tensor(mean_t[:], ps_sel[0][:], b2e_sb[:, 0:512],
                                    op=ALU.add)
            nc.vector.tensor_tensor(std_t[:], ps_sel[1][:], b2e_sb[:, 512:E2],
                                    op=ALU.add)
            pmt = psA.tile([128, 2 * DC], F32, tag="rbps", bufs=1, name="pmt")
            for j in range(DC):
                jsl = slice(j * 128, (j + 1) * 128)
                nc.tensor.matmul(pmt[:, j:j + 1], mean_t[0:1, jsl],
                                 onesf[0:1, 0:1], is_transpose=True,
                                 skip_group_check=True)
                nc.tensor.matmul(pmt[:, DC + j:DC + j + 1], std_t[0:1, jsl],
                                 onesf[0:1, 0:1], is_transpose=True,
                                 skip_group_check=True)
            nc.vector.tensor_copy(stc[:], pmt[:, DC:2 * DC])
            nc.vector.tensor_copy(msrall[:], pmt[:])

        gcols = tailrows.tile([128, 2 * DC], F32, tag="gcols")

        def emit_pair_stats(j, gsqp):
            """per-pair global-norm partials as [128,1] columns (DVE only)."""
            nc.vector.reduce_sum(gcols[:, j:j + 1], outT[j][:],
                                 axis=mybir.AxisListType.X)
            sq = gsqp.tile([128, TQ], BF, tag="gsq", bufs=2, name="sq")
            nc.vector.tensor_tensor(sq[:], outT[j][:], outT[j][:],
                                    op=ALU.mult)
            nc.vector.reduce_sum(gcols[:, DC + j:DC + j + 1], sq[:],
                                 axis=mybir.AxisListType.X)

        with tc.tile_pool(name="ep", bufs=6) as epool, \
             tc.tile_pool(name="gsq1", bufs=1) as gsqp, \
             tc.tile_pool(name="psA", bufs=1, space="PSUM") as psA:
            for h in range(H):
                po = psA.tile([65, TQ], F32, tag="po", bufs=3, name="po")
                hp, ho = h // 2, (h % 2) * 64
                exs = []
                po_emitted = 0

                def emit_po(kt):
                    nc.tensor.matmul(po[:], vt[kt][:, h * 65:(h + 1) * 65],
                                     exs[kt // 2][:, (kt % 2) * TQ:
                                                   (kt % 2) * TQ + TQ],
                                     start=(kt == 0), stop=(kt == KT - 1),
                                     skip_group_check=True)

                for p in range(KT // 2):
                    ab = epool.tile([128, 2 * TQ], F32, tag="ab", name="ab")
                    pst = psA.tile([128, 2 * TQ], F32, tag="stps", bufs=2,
                                   name="pst")
                    for kk in range(2):
                        kt = 2 * p + kk
                        nc.tensor.matmul(
                            pst[:, kk * TQ:kk * TQ + TQ],
                            kTp[hp][ho:ho + 64, kt * 128:(kt + 1) * 128],
                            qTp[hp][ho:ho + 64, :],
                            start=True, stop=True, skip_group_check=True)
                    if p == KT // 2 - 1:
                        # last pair: both abs on ACT (engine balance)
                        nc.scalar.activation(ab[:, 0:TQ], pst[:, 0:TQ],
                                             AF.Abs)
                        nc.scalar.activation(ab[:, TQ:2 * TQ],
                                             pst[:, TQ:2 * TQ], AF.Abs)
                    else:
                        # one [128,1024] int-AND covers the whole pair
                        nc.vector.tensor_scalar(ab[:].bitcast(I32),
                                                pst[:].bitcast(I32),
                                                0x7FFFFFFF, None,
                                                op0=ALU.bitwise_and)
                    ex = epool.tile([128, 2 * TQ], BF, tag="ex", name="ex")
                    nc.scalar.activation(ex[:], ab[:], AF.Exp)
                    exs.append(ex)
                    if p >= 1:
                        emit_po(po_emitted)
                        po_emitted += 1
                        emit_po(po_emitted)
                        po_emitted += 1
                while po_emitted < KT:
                    emit_po(po_emitted)
                    po_emitted += 1
                # denom row -> dnr[h]; softmax-normalize straight from PSUM
                j, hh = h // 2, h % 2
                nc.vector.tensor_copy(dnr[h][:], po[64:65, :])
                prb = psA.tile([64, TQ], F32, tag="rbps", bufs=1, name="prb")
                nc.tensor.matmul(prb[:], ones_row[0:1, 0:64], dnr[h][:],
                                 start=True, stop=True)
                rb_sb = gsqp.tile([64, TQ], F32, tag="rbsb", bufs=2,
                                  name="rb_sb")
                nc.vector.reciprocal_approx_fast(rb_sb[:], prb[:])
                nc.vector.tensor_tensor(outT[j][hh * 64:(hh + 1) * 64, :],
                                        po[0:64, :], rb_sb[:], op=ALU.mult)
                if h == 3:
                    emit_pair_stats(0, gsqp)
                if h == 5:
                    emit_pair_stats(1, gsqp)
            emit_pair_stats(2, gsqp)
            emit_pair_stats(3, gsqp)
            pgs = psA.tile([1, 2 * DC], F32, tag="rbps", bufs=1, name="pgs")
            nc.tensor.matmul(pgs[:], f32(ones_col[:]), gcols[:],
                             start=True, stop=True)
            srow = rowpool.tile([1, 2], F32, tag="srow")
            nc.vector.reduce_sum(srow[:, 0:1], pgs[0:1, 0:DC],
                                 axis=mybir.AxisListType.X)
            nc.vector.reduce_sum(srow[:, 1:2], pgs[0:1, DC:2 * DC],
                                 axis=mybir.AxisListType.X)
            nc.sync.dma_start(ar2_in_d[:], srow[:])
            # FiLM-row processing here: AR1 has had the whole attention
            # span to land, and the PE queue is no longer hostage to it.
            emit_t_processing(psA)

        nc.gpsimd.collective_compute(
            "AllReduce", ALU.add,
            replica_groups=[[2 * i, 2 * i + 1] for i in range(N_CORES // 2)],
            ins=[ar2_in_d.opt()], outs=[ar2_out_d.opt()])
        ar2_sb = rowpool.tile([1, 2], F32R, tag="ar2sb")
        nc.sync.dma_start(ar2_sb[:], ar2_out_d[:].bitcast(F32R))

        # ---- tail --------------------------------------------------------
        # out1 = (outT-mu)/sd*std_t + mean_t ; y = wo^T out1 + bo.  Since
        # mu/sd are POST-AR2 scalars, decompose:
        #   y = inv_sd*A + (cb (x) ones),  A = (wo . std_t-rows)^T outT,
        #   cb = wo^T mean_t + bo - mu*inv_sd*(wo^T std_t) = r1b - g1*r2
        # and m1^T y = inv_sd*B + (m1^T cb)(x)ones,  B = m1^T A.
        # A, B, r12 = wo^T[mean|std], m1c12 = m1^T[r1b|r2] are all AR2-free
        # and overlap the AllReduce; only scalar fixups + LN2 + gelu + m2
        # remain on the critical path.
        with tc.tile_pool(name="gsqp", bufs=2) as gsqp2, \
             tc.tile_pool(name="psP", bufs=1, space="PSUM") as psP:
            # -- pre-AR2 work --
            wo2 = [attnpool.tile([128, TQ], BF, tag=f"kTp{j}",
                                 name=f"wo2_{j}") for j in range(DC)]
            for j in range(DC):
                nc.vector.tensor_scalar(wo2[j][:], wo_t[j][:],
                                        stc[:, j:j + 1], None, op0=ALU.mult)
            A = [attnpool.tile([128, TQ], BF, tag=f"qTp{j}", name=f"A{j}")
                 for j in range(DC)]
            for mo in range(DC):
                msl = slice(mo * 128, (mo + 1) * 128)
                pp = psP.tile([128, TQ], F32, tag="pk2", bufs=3, name="pp")
                for j in range(DC):
                    nc.tensor.matmul(pp[:], wo2[j][:, msl], outT[j][:],
                                     start=(j == 0), stop=(j == DC - 1))
                nc.scalar.activation(A[mo][:], pp[:], AF.Identity)
            r12c = tailrows.tile([128, 2 * DC], F32, tag="r12c")
            for mo in range(DC):
                msl = slice(mo * 128, (mo + 1) * 128)
                pr = psP.tile([128, 2], F32, tag="pr12", bufs=2, name="pr")
                for j in range(DC):
                    nc.tensor.matmul(pr[:], wo_t[j][:, msl],
                                     msrall[:, j:j + DC + 1:DC],
                                     start=(j == 0), stop=(j == DC - 1))
                nc.vector.tensor_copy(r12c[:, 2 * mo:2 * mo + 2], pr[:])
            w12all = tailrows.tile([128, 2 * DC], BF, tag="w12all")
            nc.vector.tensor_copy(w12all[:], r12c[:])
            nc.vector.tensor_tensor(w12all[:, 0:2 * DC:2],
                                    w12all[:, 0:2 * DC:2], bias_sb["bo"],
                                    op=ALU.add)
            m1c12 = tailrows.tile([128, 2 * DC], F32, tag="m1c12")
            for mo in range(DC):
                msl = slice(mo * 128, (mo + 1) * 128)
                pr = psP.tile([128, 2], F32, tag="pr12", bufs=2, name="pr")
                for j in range(DC):
                    nc.tensor.matmul(pr[:], m1_t[j][:, msl],
                                     w12all[:, 2 * j:2 * j + 2],
                                     start=(j == 0), stop=(j == DC - 1))
                nc.vector.tensor_copy(m1c12[:, 2 * mo:2 * mo + 2], pr[:])
            Bm = [attnpool.tile([128, TQ], BF, tag=f"vt{2 * j}",
                                name=f"Bm{j}") for j in range(DC)]
            for mo in range(DC):
                msl = slice(mo * 128, (mo + 1) * 128)
                pp = psP.tile([128, TQ], F32, tag="pk2", bufs=3, name="pp")
                for j in range(DC):
                    nc.tensor.matmul(pp[:], m1_t[j][:, msl], A[j][:],
                                     start=(j == 0), stop=(j == DC - 1))
                nc.scalar.activation(Bm[mo][:], pp[:], AF.Identity)

            # -- post-AR2 scalars --
            ps_st = psP.tile([128, 2], F32, tag="pr12", bufs=2, name="ps_st")
            nc.tensor.matmul(ps_st[:], ones_row[:], ar2_sb[:],
                             start=True, stop=True)
            mu = scw("mu")
            nc.vector.tensor_scalar_mul(mu, ps_st[:, 0:1], 1.0 / NEL)
            smu = scw("smu")
            nc.vector.tensor_tensor(smu, ps_st[:, 0:1], mu, op=ALU.mult)
            var1 = scw("var1")
            nc.vector.tensor_tensor(var1, ps_st[:, 1:2], smu, op=ALU.subtract)
            var1s = scw("var1s")
            nc.vector.tensor_scalar_mul(var1s, var1, 1.0 / (NEL - 1.0))
            sd_g = scw("sd_g")
            nc.scalar.activation(sd_g, var1s, AF.Sqrt)
            inv_sd = scw("inv_sd")
            nc.vector.reciprocal(inv_sd, sd_g)
            g1 = scw("g1")
            nc.vector.tensor_tensor(g1, mu, inv_sd, op=ALU.mult)
            # cb = r1 + bo - g1*r2 ; m1cc = m1c1b - g1*m1c2 (cols, f32)
            cb = tailrows.tile([128, DC], F32, tag="cb")
            nc.vector.tensor_scalar(cb[:], r12c[:, 1:2 * DC:2], g1, None,
                                    op0=ALU.mult)
            nc.vector.tensor_tensor(cb[:], r12c[:, 0:2 * DC:2], cb[:],
                                    op=ALU.subtract)
            nc.vector.tensor_tensor(cb[:], cb[:], bias_sb["bo"], op=ALU.add)
            m1cc = tailrows.tile([128, DC], F32, tag="m1cc")
            nc.vector.tensor_scalar(m1cc[:], m1c12[:, 1:2 * DC:2], g1, None,
                                    op0=ALU.mult)
            nc.vector.tensor_tensor(m1cc[:], m1c12[:, 0:2 * DC:2], m1cc[:],
                                    op=ALU.subtract)

            # -- y (for LN2 stats only) + stats --
            y = [attnpool.tile([128, TQ], BF, tag=f"yst{j}", name=f"y{j}")
                 for j in range(DC)]
            ps_s2 = psP.tile([1, TQ], F32, tag="prow2", bufs=2, name="ps_s2")
            ps_q2 = psP.tile([1, TQ], F32, tag="prow2", bufs=2, name="ps_q2")
            for j in range(DC):
                nc.vector.tensor_scalar(y[j][:], A[j][:], inv_sd,
                                        cb[:, j:j + 1],
                                        op0=ALU.mult, op1=ALU.add)
                sq = gsqp2.tile([128, TQ], BF, tag="gsq", name="sq")
                nc.vector.tensor_tensor(sq[:], y[j][:], y[j][:],
                                        op=ALU.mult)
                nc.tensor.matmul(ps_s2[:], ones_col_bf[:], y[j][:],
                                 start=(j == 0), stop=(j == DC - 1),
                                 skip_group_check=True)
                nc.tensor.matmul(ps_q2[:], ones_col_bf[:], sq[:],
                                 start=(j == 0), stop=(j == DC - 1),
                                 skip_group_check=True)
            m2row = tailrows.tile([1, TQ], F32R, tag="m2row")
            nc.vector.tensor_scalar_mul(m2row[:], ps_s2[:], 1.0 / D)
            var2 = srt(TQ, "var2")
            nc.vector.tensor_scalar(var2, ps_q2[:], 1.0 / D, 1e-5,
                                    op0=ALU.mult, op1=ALU.add)
            msq2 = srt(TQ, "msq2")
            nc.scalar.activation(msq2, f32(m2row[:]), AF.Square)
            nc.vector.tensor_tensor(var2, var2, msq2, op=ALU.subtract)
            sd2 = tailrows.tile([1, TQ], F32R, tag="sd2")
            nc.scalar.activation(sd2[:], var2, AF.Sqrt)
            inv2 = tailrows.tile([1, TQ], F32, tag="inv2")
            nc.vector.reciprocal_approx_fast(inv2[:], f32(sd2[:]))
            pib = psP.tile([128, TQ], F32, tag="pk2", bufs=3, name="pib")
            nc.tensor.matmul(pib[:], onesf[0:1, :], inv2[:],
                             start=True, stop=True)
            i2b = tailrows.tile([128, TQ], F32, tag="i2b")
            nc.scalar.activation(i2b[:], pib[:], AF.Identity)

            # gelu(i2b * (inv_sd*B + m1cc + nws3(x)m2row [+ pb3(x)sd2]))
            g = [attnpool.tile([128, TQ], BF, tag=f"gst{j}",
                               name=f"g{j}") for j in range(DC)]
            for mo in range(DC):
                msl = slice(mo * 128, (mo + 1) * 128)
                pp = psP.tile([128, TQ], F32, tag="pk2", bufs=3, name="pp")
                nc.tensor.matmul(pp[:], nws_sb[3][:, msl], m2row[:],
                                 start=True, stop=not has_bias)
                if has_bias:
                    nc.tensor.matmul(pp[:], pb_sb[3][:, msl], sd2[:],
                                     start=False, stop=True)
                t1 = gsqp2.tile([128, TQ], BF, tag="t1", name="t1")
                nc.vector.tensor_scalar(t1[:], Bm[mo][:], inv_sd,
                                        m1cc[:, mo:mo + 1],
                                        op0=ALU.mult, op1=ALU.add)
                t2 = gsqp2.tile([128, TQ], F32R, tag="gsq2", name="t2")
                nc.vector.tensor_tensor(t2[:], t1[:], pp[:], op=ALU.add)
                gin = gsqp2.tile([128, TQ], F32R, tag="gin", name="gin")
                nc.vector.tensor_tensor(gin[:], f32(t2[:]), i2b[:],
                                        op=ALU.mult)
                nc.scalar.activation(g[mo][:], f32(gin[:]), _GELU_FUNC)

            yf = [attnpool.tile([128, TQ], F32, tag=f"vt{2 * j + 1}",
                                name=f"yf{j}") for j in range(DC)]
            for mo in range(DC):
                pp = psP.tile([128, TQ], F32, tag="pk2", bufs=3, name="pp")
                for j in range(DC):
                    nc.tensor.matmul(pp[:], m2_t[j][:, mo * 128:(mo + 1) * 128],
                                     g[j][:], start=(j == 0),
                                     stop=(j == DC - 1))
                nc.scalar.activation(yf[mo][:], pp[:], AF.Identity,
                                     bias=bias_sb["b2"][:, mo:mo + 1])
                nc.sync.dma_start(out[mo * 128:(mo + 1) * 128, :], yf[mo][:])


_NC_CACHE = {}
_GELU_FUNC = AF.Gelu


def _get_nc(gelu_mode="hw", has_bias=False):
    key = (gelu_mode, has_bias)
    if key not in _NC_CACHE:
        _NC_CACHE[key] = _build_nc(gelu_mode, has_bias)
    return _NC_CACHE[key]


def _prep_in_maps(inputs):
    f = lambda k: np.ascontiguousarray(np.asarray(inputs[k], dtype=np.float32))
    diff, con, temb = f("diff_features"), f("con_features"), f("time_emb")
    g_d, b_d = f("ln_diff_g"), f("ln_diff_b")
    g_c, b_c = f("ln_con_g"), f("ln_con_b")
    wq_, wk_, wv_ = f("wq"), f("wk"), f("wv")
    wo_, bo_ = f("w_out"), f("b_out")
    w1e_, b1e_, w2e_, b2e_ = f("w_emd1"), f("b_emd1"), f("w_emd2"), f("b_emd2")
    gm, bm = f("mlp_ln_g"), f("mlp_ln_b")
    m1_, mb1_, m2_, mb2_ = f("mlp_w1"), f("mlp_b1"), f("mlp_w2"), f("mlp_b2")

    wq_f = g_d[:, None] * wq_
    wk_f = g_c[:, None] * wk_
    wv_f = (g_c[:, None] * wv_) / 8.0      # fold softmax /sqrt(DH)
    bq_v = b_d @ wq_
    bk_v = b_c @ wk_
    bv_v = (b_c @ wv_) / 8.0
    m1_f = gm[:, None] * m1_
    mb1_f = mb1_ + bm @ m1_
    has_bias = bool(np.any(bq_v) or np.any(bk_v) or np.any(bv_v)
                    or np.any(mb1_f))
    nws = -np.stack([wq_f.sum(0), wk_f.sum(0), wv_f.sum(0), m1_f.sum(0)])
    pbias = np.stack([bq_v, bk_v, bv_v, mb1_f])
    flip = (-np.arange(NT)) % NT

    def br(v):
        return np.ascontiguousarray(v.reshape(DC, 128).T)

    import ml_dtypes
    BF16_KEYS = {"xq", "xkv", "xv", "wkqv", "womm"}

    def bf(v):
        return np.ascontiguousarray(
            np.asarray(v, np.float32).astype(ml_dtypes.bfloat16))

    common = {
        "wkqv": bf(np.concatenate([wk_f, wq_f, wv_f], axis=1)),
        "womm": bf(np.concatenate([wo_, m1_f, m2_], axis=1)),
        "nws": nws, "pbias": pbias,
        "emb": np.ascontiguousarray(temb.T),
        "bo": br(bo_), "b2": br(mb2_),
        "b2e": b2e_.reshape(1, E2),
        "ones8": np.ones((128, H), np.float32),
    }
    in_maps = []
    for c in range(N_CORES):
        b, off = c // 2, (c % 2) * TQ
        sel = np.zeros((B, 1), np.float32)
        sel[b, 0] = 1.0
        m = dict(common)
        m.update({
            "xq": bf(diff[b, off:off + TQ].T),
            "xkv": bf(con[b].T),
            "xv": bf(con[b][flip].T),
            "w1e": np.ascontiguousarray(w1e_[:, c * 128:(c + 1) * 128]),
            "w2e": np.ascontiguousarray(w2e_[c * 128:(c + 1) * 128, :]),
            "b1e": np.ascontiguousarray(b1e_[c * 128:(c + 1) * 128]
                                        .reshape(128, 1)),
            "sel4": sel,
        })
        in_maps.append({k: (np.ascontiguousarray(v) if k in BF16_KEYS
                            else np.ascontiguousarray(
                                np.asarray(v).astype(np.float32)))
                        for k, v in m.items()})
    return in_maps, has_bias


def _assemble(results):
    outp = np.empty((B, NT, D), np.float32)
    for c in range(N_CORES):
        b, off = c // 2, (c % 2) * TQ
        outp[b, off:off + TQ, :] = results[c]["out"].T
    return outp


def kernel(**inputs):
    in_maps, has_bias = _prep_in_maps(inputs)
    nc = _get_nc("hw", has_bias)
    res = run_bass_kernel_spmd(nc, in_maps, core_ids=list(range(N_CORES)))
    return _assemble(res.results)

